# revision 16
# baseline (speedup 1.0000x reference)
"""Trainium2 Bass kernel for nn_DataEmbedding_cycle_pos.

Math (B=16, T=2048, N=8, D=512), out[b,t,:] =
    conv(x)               Conv1d(N->D, k=3, circular)        -> matmul K=24
  + temporal(x_mark)      sum of 4 fixed-table lookups; host precomputes the
                          28-row onehot so it's onehot28 @ R4  -> matmul K=28
  + cycle-positional      periods = clip(T/freq[argmax |rfft|], 1, T); for
                          T=2048 the period is 2048 unless the argmax is
                          exactly the Nyquist bin (then 1.0).  Per (b,n) only
                          the bit "is Nyquist the strict max" matters:
                            cyc[b] = (1-cnt/8)*postab + (cnt/8)*row01
                          cnt = #Nyquist-max series in batch b.
  The row01 (odd-column ones) term folds into the onehot matmul rows since
  sum(onehot) == 4 exactly:  R4 + (cnt/32)*odd.

Sharding: batch-parallel (2 batches/core).  The |rfft|^2 argmax test is
computed per core for its OWN 16 series (no collectives) via a
quarter-size DFT: double time-fold (u = 0..511 in 4 row-chunks, plus a
K=1 residual matmul for u=512) x frequency-parity split, in bf16 matmuls.

Engine/DMA layout (each dma_start costs ~700ns of issuing-engine time, so
inputs are coalesced host-side into 7 transfers):
  - gpsimd SWDGE (436 GB/s ring): the two big tables (tabs, postab)
  - scalar HWDGE: the 7 coalesced small inputs, ACT squares, batch-1
    PSUM->SBUF drains (pairs of tiles, [128,1024])
  - vector: FFT glue, batch-0 fused drain (STT from PSUM), batch-1
    prescale a1*postab and bf16 adds (2x mode)
  - sync HWDGE: 8 output DMAs ([BPC,128,NT*D] bf16 layout, 4KB packets)
"""
import sys, os

sys.path.insert(0, "/opt/trn_rl_repo")
import numpy as np
import ml_dtypes

import concourse.bass as bass
import concourse.bacc as bacc
import concourse.mybir as mybir
import concourse.tile as tile
from concourse.bass_utils import run_bass_kernel_spmd

B, T, N, D = 16, 2048, 8, 512
NCORES = 8
BPC = B // NCORES          # batches per core
SPC = BPC * N              # series per core (16)
NT = T // 128              # 128-row time tiles per batch
KCONV = 3 * N              # 24 conv rows
KHOT = 28                  # 4 features x 7 index values
KTOT = KCONV + KHOT        # 52
NYQ = T // 2               # 1024
UCH = 4                    # u chunks of 128 covering u=0..511
FEW = 516                  # even-parity freq cols (Nyquist first, 3 pad)
CHW = 2 * FEW + 2 * 512    # 2056 table cols per chunk: ce|se|co|so

F32 = mybir.dt.float32
BF16 = mybir.dt.bfloat16
BF = ml_dtypes.bfloat16

TRACE = False
TRACE_DIR = None

_cache = {}


# ----------------------------------------------------------------- constants
def _div_term():
    # mirror reference: exp(arange(0,512,2) * (-ln 10000 / 512)) in f32
    return np.exp(
        np.arange(0, D, 2, dtype=np.float32) * np.float32(-np.log(10000.0) / D)
    ).astype(np.float32)


def _fixed_rows(nrows):
    pos = np.arange(nrows, dtype=np.float32)[:, None]
    ang = (pos * _div_term()[None, :]).astype(np.float32)
    tab = np.zeros((nrows, D), dtype=np.float32)
    tab[:, 0::2] = np.sin(ang)
    tab[:, 1::2] = np.cos(ang)
    return tab


def _host_constants():
    c = {}
    postab = _fixed_rows(T)  # [2048, 512]
    # SBUF layout [128(tt), 16 tiles * 512]
    c["postab"] = np.ascontiguousarray(
        postab.reshape(NT, 128, D).transpose(1, 0, 2).reshape(128, NT * D)
    ).astype(BF)
    r7 = _fixed_rows(7)
    odd = np.zeros((D,), dtype=np.float32)
    odd[1::2] = 1.0
    r4 = np.tile(r7, (4, 1)).astype(np.float32)
    odd28 = np.tile(odd[None, :], (KHOT, 1)).astype(np.float32)
    c["r4odd"] = np.ascontiguousarray(
        np.concatenate([r4, odd28], axis=1))           # [28, 1024] f32

    # quarter DFT tables (double time-fold, frequencies split by parity):
    # chunk-major packed [128(tt), 4 ch * (ce|se|co|so)] covering u=0..511;
    # the u=512 row is a separate residual (ce | so only; sin(pi*k/2)=0 for
    # even k and cos(pi*k/2)=0 for odd k kill se/co), packed into `misc`.
    w = 2.0 * np.pi / T
    ke = np.arange(0, NYQ + 1, 2, dtype=np.float64)            # 513 even
    ko = np.arange(1, NYQ, 2, dtype=np.float64)                # 512 odd
    kep = np.concatenate([ke[512:], ke[:512]])                 # nyq first
    kev = np.zeros(FEW, dtype=np.float64); kev[:513] = kep
    kevm = np.zeros(FEW); kevm[:513] = 1.0

    uu = np.arange(UCH * 128, dtype=np.float64)                # u = 0..511
    blocks = []
    for ch in range(UCH):
        ur = uu[ch * 128:(ch + 1) * 128]
        ce = np.cos(w * np.outer(ur, kev)) * kevm[None, :]
        se = np.sin(w * np.outer(ur, kev)) * kevm[None, :]
        co = np.cos(w * np.outer(ur, ko))
        so = np.sin(w * np.outer(ur, ko))
        blocks.append(np.concatenate([ce, se, co, so], axis=1))
    c["tabs"] = np.concatenate(blocks, axis=1).astype(np.float32).astype(BF)

    ce5 = np.cos(w * 512.0 * kev) * kevm
    so5 = np.sin(w * 512.0 * ko)
    c["tab512"] = np.concatenate([ce5, so5]).astype(np.float32).astype(BF)

    # batch indicator for the cnt matmul: ind2[s, i*128+p] = (s//8 == i)
    s_batch = np.arange(SPC) // N
    cols = [np.tile((s_batch == i).astype(np.float32)[:, None], (1, 128))
            for i in range(BPC)]
    c["ind2"] = np.concatenate(cols, axis=1).astype(BF)        # [16, 256]
    return c


# ------------------------------------------------------------------- program
def _build_nc():
    nc = bacc.Bacc("TRN2", target_bir_lowering=False, debug=False,
                   num_devices=NCORES)

    def din(name, shape, dt):
        return nc.dram_tensor(name, shape, dt, kind="ExternalInput").ap()

    MW = 2 * SPC + FEW + 512                      # misc row: xq5 | tab512
    xq = din("xq", [128, 4 * UCH * SPC], BF16)    # butterfly ops a|b|c|d
    misc = din("misc", [1, MW], BF16)
    tabs = din("tabs", [128, UCH * CHW], BF16)    # chunk-major DFT tables
    ltf = din("ltf", [BPC, KTOT, T], BF16)        # onehot28 + 3 conv shifts
    w24 = din("w24", [KCONV, D], BF16)
    r4odd = din("r4odd", [KHOT, 2 * D], F32)
    ind2 = din("ind2", [SPC, BPC * 128], BF16)
    postab = din("postab", [128, NT * D], BF16)
    out = nc.dram_tensor("out", [BPC, 128, NT * D], BF16,
                         kind="ExternalOutput").ap()

    with tile.TileContext(nc) as tc:
        with (
            tc.tile_pool(name="consts", bufs=1) as cpool,
            tc.tile_pool(name="fwork", bufs=1) as fpool,
            tc.tile_pool(name="fpsum", bufs=1, space="PSUM") as fpsum,
            tc.tile_pool(name="cpsum", bufs=1, space="PSUM") as cpsum,
            tc.tile_pool(name="mpsum", bufs=2, space="PSUM") as mpsum,
            tc.tile_pool(name="batch", bufs=1) as bpool,
            tc.tile_pool(name="outp", bufs=2) as opool,
            tc.tile_pool(name="cppool", bufs=2) as cppool,
        ):
            UW = UCH * SPC  # 64 butterfly cols

            # -------- SBUF tiles
            xq_sb = fpool.tile([128, 4 * UW], BF16, tag="xq")
            misc_sb = fpool.tile([1, MW], BF16, tag="misc")
            tabs_sb = cpool.tile([128, UCH * CHW], BF16, tag="tabs")
            ind2_sb = cpool.tile([SPC, BPC * 128], BF16, tag="ind2")
            r4odd_sb = cpool.tile([KHOT, 2 * D], F32, tag="r4odd")
            postab_sb = cpool.tile([128, NT * D], BF16, tag="postab")
            lts = [bpool.tile([KTOT, T], BF16, tag=f"lt{i}", name=f"lt{i}")
                   for i in range(BPC)]
            rhss = [bpool.tile([KTOT, D], BF16, tag=f"rhs{i}", name=f"rhs{i}")
                    for i in range(BPC)]
            xq5_sb = misc_sb[:, 0:2 * SPC]
            t512_sb = misc_sb[:, 2 * SPC:]

            # -------- single scalar HWDGE ring in strict priority order
            # (input reads share ~310 GB/s across all queues, so one
            # ordered ring beats parallel rings)
            nc.scalar.dma_start(xq_sb[:], xq)
            nc.scalar.dma_start(misc_sb[:], misc)
            for ch in range(UCH):
                nc.scalar.dma_start(tabs_sb[:, ch * CHW:(ch + 1) * CHW],
                                    tabs[:, ch * CHW:(ch + 1) * CHW])
            nc.scalar.dma_start(lts[0][:], ltf[0])
            nc.scalar.dma_start(lts[1][:], ltf[1])
            nc.scalar.dma_start(ind2_sb[:], ind2)
            nc.scalar.dma_start(r4odd_sb[:], r4odd)
            nc.scalar.dma_start(rhss[0][KHOT:KTOT, :], w24)
            nc.scalar.dma_start(rhss[1][KHOT:KTOT, :], w24)
            for q in range(4):
                nc.scalar.dma_start(
                    postab_sb[:, q * 4 * D:(q + 1) * 4 * D],
                    postab[:, q * 4 * D:(q + 1) * 4 * D])

            # -------- FFT phase: butterflies (vector)
            xa_sb = xq_sb[:, 0 * UW:1 * UW]
            xb_sb = xq_sb[:, 1 * UW:2 * UW]
            xc_sb = xq_sb[:, 2 * UW:3 * UW]
            xd_sb = xq_sb[:, 3 * UW:4 * UW]
            ab = fpool.tile([128, UW], BF16, tag="ab")
            nc.vector.tensor_add(ab[:], xa_sb, xb_sb)
            amb = fpool.tile([128, UW], BF16, tag="amb")
            nc.vector.tensor_sub(amb[:], xa_sb, xb_sb)
            cd = fpool.tile([128, UW], BF16, tag="cd")
            nc.vector.tensor_add(cd[:], xc_sb, xd_sb)
            cmd = fpool.tile([128, UW], BF16, tag="cmd")
            nc.vector.tensor_sub(cmd[:], xc_sb, xd_sb)
            pce = fpool.tile([128, UW], BF16, tag="pce")
            nc.vector.tensor_add(pce[:], ab[:], cd[:])
            pco = fpool.tile([128, UW], BF16, tag="pco")
            nc.vector.tensor_sub(pco[:], ab[:], cd[:])
            pse = fpool.tile([128, UW], BF16, tag="pse")
            nc.vector.tensor_sub(pse[:], amb[:], cmd[:])
            pso = fpool.tile([128, UW], BF16, tag="pso")
            nc.vector.tensor_add(pso[:], amb[:], cmd[:])

            # -------- FFT matmuls: psum bank A rows re_e/re_o/im_e/im_o at
            # bases 0/32/64/96; Nyquist+pad tail in ps_tail
            ps_main = fpsum.tile([112, 512], F32, tag="psmain")
            ps_tail = fpsum.tile([48, 4], F32, tag="pstail")
            opnds = {"ce": pce, "co": pco, "se": pse, "so": pso}
            off = {"ce": 0, "se": FEW, "co": 2 * FEW, "so": 2 * FEW + 512}
            for ch in range(UCH):
                st = (ch == 0)
                base = ch * CHW
                for nm, tb in (("ce", 0), ("se", 32)):
                    nc.tensor.matmul(ps_tail[tb:tb + SPC, :],
                                     opnds[nm][:, ch * SPC:(ch + 1) * SPC],
                                     tabs_sb[:, base + off[nm]:base + off[nm] + 4],
                                     start=st, stop=(nm == "se" and ch == UCH - 1),
                                     tile_position=(0, tb),
                                     skip_group_check=True)
                for pb, nm in ((0, "ce"), (32, "co"), (64, "se"), (96, "so")):
                    lhs = opnds[nm][:, ch * SPC:(ch + 1) * SPC]
                    if nm in ("ce", "se"):
                        cols = tabs_sb[:, base + off[nm] + 4:base + off[nm] + FEW]
                    else:
                        cols = tabs_sb[:, base + off[nm]:base + off[nm] + 512]
                    sp = (ch == UCH - 1) and pb in (32, 64)
                    nc.tensor.matmul(ps_main[pb:pb + SPC, :], lhs, cols,
                                     start=st, stop=sp,
                                     tile_position=(0, pb),
                                     skip_group_check=True)
            # u=512 residual: re_e += pce512*cos(pi k/2), im_o += pso512*sin
            nc.tensor.matmul(ps_tail[0:SPC, :], xq5_sb[:, 0:SPC],
                             t512_sb[:, 0:4],
                             start=False, stop=True, tile_position=(0, 0),
                             skip_group_check=True)
            nc.tensor.matmul(ps_main[0:SPC, :], xq5_sb[:, 0:SPC],
                             t512_sb[:, 4:FEW],
                             start=False, stop=True, tile_position=(0, 0),
                             skip_group_check=True)
            nc.tensor.matmul(ps_main[96:96 + SPC, :], xq5_sb[:, SPC:2 * SPC],
                             t512_sb[:, FEW:FEW + 512],
                             start=False, stop=True, tile_position=(0, 96),
                             skip_group_check=True)

            # -------- |X|^2: one big ACT square over all 4 DFT groups,
            # then quarter-aligned DVE adds (re^2 + im^2)
            mag_e = fpool.tile([SPC, 512], F32, tag="mag_e")
            mag_o = fpool.tile([SPC, 512], F32, tag="mag_o")
            mag_t = fpool.tile([SPC, 4], F32, tag="mag_t")
            sqa = fpool.tile([SPC, 512], F32, tag="sqa")
            sqb = fpool.tile([SPC, 512], F32, tag="sqb")
            sqc = fpool.tile([SPC, 4], F32, tag="sqc")
            nc.scalar.square(mag_e[:], ps_main[0:SPC, :])
            nc.scalar.square(sqa[:], ps_main[64:64 + SPC, :])
            nc.vector.tensor_add(mag_e[:], mag_e[:], sqa[:])
            nc.scalar.square(mag_o[:], ps_main[32:32 + SPC, :])
            nc.scalar.square(sqb[:], ps_main[96:96 + SPC, :])
            nc.vector.tensor_add(mag_o[:], mag_o[:], sqb[:])
            nc.scalar.square(mag_t[:], ps_tail[0:SPC, :])
            nc.scalar.square(sqc[:], ps_tail[32:32 + SPC, :])
            nc.vector.tensor_add(mag_t[:], mag_t[:], sqc[:])

            # strict >: Nyquist wins only if greater than every earlier bin
            lm_e = fpool.tile([SPC, 1], F32, tag="lm_e")
            nc.vector.reduce_max(lm_e[:], mag_e[:], axis=mybir.AxisListType.X)
            lm_o = fpool.tile([SPC, 1], F32, tag="lm_o")
            nc.vector.reduce_max(lm_o[:], mag_o[:], axis=mybir.AxisListType.X)
            lm_t = fpool.tile([SPC, 1], F32, tag="lm_t")
            nc.vector.reduce_max(lm_t[:], mag_t[:, 1:4],
                                 axis=mybir.AxisListType.X)
            lm2 = fpool.tile([SPC, 1], F32, tag="lm2")
            nc.vector.tensor_max(lm2[:], lm_e[:], lm_o[:])
            lmax = fpool.tile([SPC, 1], F32, tag="lmax")
            nc.vector.tensor_max(lmax[:], lm2[:], lm_t[:])
            isn = fpool.tile([SPC, 1], BF16, tag="isn")
            nc.vector.tensor_tensor(isn[:], mag_t[:, 0:1], lmax[:],
                                    op=mybir.AluOpType.is_gt)

            a_vecs, bq_vecs = [], []
            for i in range(BPC):
                ps_cnt = cpsum.tile([128, 1], F32, tag="pscnt")
                nc.tensor.matmul(ps_cnt[:], ind2_sb[:, i * 128:(i + 1) * 128],
                                 isn[:], start=True, stop=True)
                a_vec = fpool.tile([128, 1], F32, tag=f"avec{i}")
                nc.vector.tensor_scalar(a_vec[:], ps_cnt[:], -0.125, 1.0,
                                        op0=mybir.AluOpType.mult,
                                        op1=mybir.AluOpType.add)
                bq_vec = fpool.tile([128, 1], F32, tag=f"bqvec{i}")
                nc.vector.tensor_scalar(bq_vec[:], ps_cnt[:], 1.0 / 32.0,
                                        None, op0=mybir.AluOpType.mult)
                a_vecs.append(a_vec)
                bq_vecs.append(bq_vec)

            # rhs: hot rows R4 + (cnt/32)*odd; conv rows copied from w24
            for i in range(BPC):
                nc.vector.scalar_tensor_tensor(
                    rhss[i][0:KHOT, :], r4odd_sb[:, D:2 * D],
                    bq_vecs[i][0:KHOT, :], r4odd_sb[:, 0:D],
                    op0=mybir.AluOpType.mult, op1=mybir.AluOpType.add)

            # prescaled a_i*postab for the copy+add drain lanes
            # (only tiles 6..15 of each batch use it)
            aposts = []
            for i in range(BPC):
                ap_t = cpool.tile([128, 10 * D], BF16, tag=f"apost{i}",
                                  name=f"apost{i}")
                nc.vector.tensor_scalar(
                    ap_t[:, 0:5 * D], postab_sb[:, 6 * D:11 * D],
                    a_vecs[i][:], None, op0=mybir.AluOpType.mult)
                nc.vector.tensor_scalar(
                    ap_t[:, 5 * D:10 * D], postab_sb[:, 11 * D:16 * D],
                    a_vecs[i][:], None, op0=mybir.AluOpType.mult)
                aposts.append(ap_t)

            # -------- main matmuls (pairs share a 2-bank psum tile) + drain
            # batches interleaved so both drain lanes run concurrently
            for g in range(NT // 4):
                for i in range(BPC):
                    ot4 = opool.tile([128, 4 * D], BF16, tag=f"ot{i}",
                                     name=f"ot{i}")
                    for h in range(2):
                        ps2 = mpsum.tile([128, 2 * D], F32, tag="ps",
                                         name="ps")
                        for s in range(2):
                            ti = g * 4 + h * 2 + s
                            nc.tensor.matmul(
                                ps2[:, s * D:(s + 1) * D],
                                lts[i][:, ti * 128:(ti + 1) * 128],
                                rhss[i][:], start=True, stop=True)
                        pr = g * 2 + h      # pair index 0..7 within batch
                        tlo = (g * 4 + h * 2) * D
                        if pr < 3:
                            # lane A: fused DVE drain straight from PSUM
                            nc.vector.scalar_tensor_tensor(
                                ot4[:, h * 2 * D:(h + 1) * 2 * D],
                                postab_sb[:, tlo:tlo + 2 * D], a_vecs[i][:],
                                ps2[:], op0=mybir.AluOpType.mult,
                                op1=mybir.AluOpType.add)
                        else:
                            # lanes B/C: ACT drains PSUM->bf16, add on
                            # gpsimd (pairs 3-5) or vector (pairs 6-7)
                            cp2 = cppool.tile([128, 2 * D], BF16, tag="cp",
                                              name="cp")
                            nc.scalar.copy(cp2[:], ps2[:])
                            alo = (pr - 3) * 2 * D
                            eng = nc.gpsimd if pr < 6 else nc.vector
                            eng.tensor_add(
                                ot4[:, h * 2 * D:(h + 1) * 2 * D],
                                aposts[i][:, alo:alo + 2 * D], cp2[:])
                    nc.sync.dma_start(
                        out[i, :, g * 4 * D:(g + 1) * 4 * D], ot4[:])
    nc.compile()
    return nc


def _get_nc():
    if "nc" not in _cache:
        _cache["nc"] = _build_nc()
    return _cache["nc"]


def _host_inputs(x, x_mark, conv_w):
    # lt rows: 0:28 onehot (hot[b,j,t] = x_mark[b,t,j//7] == j%7),
    #          28:52 three circular shifts of x^T
    xm = np.asarray(x_mark).astype(np.int64)               # [16, 2048, 4]
    j = np.arange(KHOT)
    hot = (xm[:, :, j // 7] == (j % 7)[None, None, :])     # [16, 2048, 28]
    hot = hot.transpose(0, 2, 1).astype(np.float32)        # [16, 28, 2048]
    xt = np.ascontiguousarray(x.transpose(0, 2, 1))        # [16, 8, 2048]
    xtp = np.concatenate([xt[:, :, -1:], xt, xt[:, :, :1]], axis=2)
    ltf = np.concatenate(
        [hot] + [xtp[:, :, k:k + T] for k in range(3)], axis=1)  # [16,52,T]
    ltf = np.ascontiguousarray(ltf).astype(BF)
    # per-core butterfly operands [tt, ch*16 + s], u = ch*128+tt (0..511)
    uu = np.arange(UCH * 128)
    mid = uu >= 1
    ai = uu
    bi = np.where(mid, (T - uu) % T, 0)
    ci = np.where(mid, NYQ - uu, NYQ)
    di = np.where(mid, NYQ + uu, 0)
    dm = mid
    quads = []
    x5 = []
    for core in range(NCORES):
        xs = x[core * BPC:(core + 1) * BPC]                # [2, 2048, 8]
        xflat = xs.transpose(1, 0, 2).reshape(T, SPC)      # [t, s]
        qs = []
        for idx, msk in ((ai, None), (bi, None), (ci, None), (di, dm)):
            arr = xflat[idx]
            if msk is not None:
                arr = arr * msk[:, None]
            qs.append(np.ascontiguousarray(
                arr.reshape(UCH, 128, SPC).transpose(1, 0, 2)
                   .reshape(128, UCH * SPC)).astype(BF))
        quads.append(np.concatenate(qs, axis=1))           # [128, 4*64]
        pce5 = xflat[512] + xflat[1536]
        pso5 = xflat[512] - xflat[1536]
        x5.append(np.concatenate([pce5, pso5]).astype(BF))
    # conv weight rows (k, n): w24[k*8+n, d] = conv_w[d, n, k]
    w24 = np.ascontiguousarray(
        conv_w.transpose(2, 1, 0).reshape(KCONV, D)).astype(BF)
    return ltf, quads, x5, w24


def make_in_maps(x, x_mark, conv_w):
    if "consts" not in _cache:
        _cache["consts"] = _host_constants()
    c = _cache["consts"]
    ltf, quads, x5, w24 = _host_inputs(x, x_mark, conv_w)
    in_maps = []
    for core in range(NCORES):
        b0 = core * BPC
        misc = np.concatenate([x5[core], c["tab512"]])[None, :]
        in_maps.append({
            "xq": quads[core],
            "misc": np.ascontiguousarray(misc),
            "tabs": c["tabs"],
            "ltf": np.ascontiguousarray(ltf[b0:b0 + BPC]),
            "w24": w24,
            "r4odd": c["r4odd"],
            "ind2": c["ind2"],
            "postab": c["postab"],
        })
    return in_maps


# -------------------------------------------------------------------- driver
def kernel(**inputs):
    x = np.asarray(inputs["x"], dtype=np.float32)          # [16, 2048, 8]
    x_mark = np.asarray(inputs["x_mark"])                  # [16, 2048, 4] int
    conv_w = np.asarray(inputs["conv_w"], dtype=np.float32)  # [512, 8, 3]

    in_maps = make_in_maps(x, x_mark, conv_w)
    nc = _get_nc()
    kw = {}
    if TRACE:
        kw = dict(trace=True, tmpdir=TRACE_DIR)
    br = run_bass_kernel_spmd(nc, in_maps, list(range(NCORES)), **kw)
    if TRACE:
        _cache["last_results"] = br

    outp = np.empty((B, T, D), dtype=np.float32)
    for core in range(NCORES):
        o = np.asarray(br.results[core]["out"]).astype(np.float32)
        o = o.reshape(BPC, 128, NT, D).transpose(0, 2, 1, 3).reshape(BPC, T, D)
        outp[core * BPC:(core + 1) * BPC] = o
    return outp


# revision 17
# speedup vs baseline: 1.0368x; 1.0368x over previous
"""Trainium2 Bass kernel for nn_DataEmbedding_cycle_pos.

Math (B=16, T=2048, N=8, D=512), out[b,t,:] =
    conv(x)               Conv1d(N->D, k=3, circular)        -> matmul K=24
  + temporal(x_mark)      sum of 4 fixed-table lookups; host precomputes the
                          28-row onehot so it's onehot28 @ R4  -> matmul K=28
  + cycle-positional      periods = clip(T/freq[argmax |rfft|], 1, T); for
                          T=2048 the period is 2048 unless the argmax is
                          exactly the Nyquist bin (then 1.0).  Per (b,n) only
                          the bit "is Nyquist the strict max" matters:
                            cyc[b] = (1-cnt/8)*postab + (cnt/8)*row01
                          cnt = #Nyquist-max series in batch b.
  The row01 (odd-column ones) term folds into the onehot matmul rows since
  sum(onehot) == 4 exactly:  R4 + (cnt/32)*odd.

Sharding: batch-parallel (2 batches/core).  The |rfft|^2 argmax test is
computed per core for its OWN 16 series (no collectives) via a
quarter-size DFT: double time-fold (u = 0..511 in 4 row-chunks, plus a
K=1 residual matmul for u=512) x frequency-parity split, in bf16 matmuls.

Engine/DMA layout (each dma_start costs ~700ns of issuing-engine time, so
inputs are coalesced host-side into 7 transfers):
  - gpsimd SWDGE (436 GB/s ring): the two big tables (tabs, postab)
  - scalar HWDGE: the 7 coalesced small inputs, ACT squares, batch-1
    PSUM->SBUF drains (pairs of tiles, [128,1024])
  - vector: FFT glue, batch-0 fused drain (STT from PSUM), batch-1
    prescale a1*postab and bf16 adds (2x mode)
  - sync HWDGE: 8 output DMAs ([BPC,128,NT*D] bf16 layout, 4KB packets)
"""
import sys, os

sys.path.insert(0, "/opt/trn_rl_repo")
import numpy as np
import ml_dtypes

import concourse.bass as bass
import concourse.bacc as bacc
import concourse.mybir as mybir
import concourse.tile as tile
from concourse.bass_utils import run_bass_kernel_spmd

B, T, N, D = 16, 2048, 8, 512
NCORES = 8
BPC = B // NCORES          # batches per core
SPC = BPC * N              # series per core (16)
NT = T // 128              # 128-row time tiles per batch
KCONV = 3 * N              # 24 conv rows
KHOT = 28                  # 4 features x 7 index values
KTOT = KCONV + KHOT        # 52
NYQ = T // 2               # 1024
UCH = 4                    # u chunks of 128 covering u=0..511
FEW = 516                  # even-parity freq cols (Nyquist first, 3 pad)
CHW = 2 * FEW + 2 * 512    # 2056 table cols per chunk: ce|se|co|so

F32 = mybir.dt.float32
BF16 = mybir.dt.bfloat16
BF = ml_dtypes.bfloat16

TRACE = False
TRACE_DIR = None

_cache = {}


# ----------------------------------------------------------------- constants
def _div_term():
    # mirror reference: exp(arange(0,512,2) * (-ln 10000 / 512)) in f32
    return np.exp(
        np.arange(0, D, 2, dtype=np.float32) * np.float32(-np.log(10000.0) / D)
    ).astype(np.float32)


def _fixed_rows(nrows):
    pos = np.arange(nrows, dtype=np.float32)[:, None]
    ang = (pos * _div_term()[None, :]).astype(np.float32)
    tab = np.zeros((nrows, D), dtype=np.float32)
    tab[:, 0::2] = np.sin(ang)
    tab[:, 1::2] = np.cos(ang)
    return tab


def _host_constants():
    c = {}
    postab = _fixed_rows(T)  # [2048, 512]
    # SBUF layout [128(tt), 16 tiles * 512]
    c["postab"] = np.ascontiguousarray(
        postab.reshape(NT, 128, D).transpose(1, 0, 2).reshape(128, NT * D)
    ).astype(BF)
    r7 = _fixed_rows(7)
    odd = np.zeros((D,), dtype=np.float32)
    odd[1::2] = 1.0
    r4 = np.tile(r7, (4, 1)).astype(np.float32)
    odd28 = np.tile(odd[None, :], (KHOT, 1)).astype(np.float32)
    c["r4odd"] = np.ascontiguousarray(
        np.concatenate([r4, odd28], axis=1))           # [28, 1024] f32

    # quarter DFT tables (double time-fold, frequencies split by parity):
    # chunk-major packed [128(tt), 4 ch * (ce|se|co|so)] covering u=0..511;
    # the u=512 row is a separate residual (ce | so only; sin(pi*k/2)=0 for
    # even k and cos(pi*k/2)=0 for odd k kill se/co), packed into `misc`.
    w = 2.0 * np.pi / T
    ke = np.arange(0, NYQ + 1, 2, dtype=np.float64)            # 513 even
    ko = np.arange(1, NYQ, 2, dtype=np.float64)                # 512 odd
    kep = np.concatenate([ke[512:], ke[:512]])                 # nyq first
    kev = np.zeros(FEW, dtype=np.float64); kev[:513] = kep
    kevm = np.zeros(FEW); kevm[:513] = 1.0

    uu = np.arange(UCH * 128, dtype=np.float64)                # u = 0..511
    blocks = []
    for ch in range(UCH):
        ur = uu[ch * 128:(ch + 1) * 128]
        ce = np.cos(w * np.outer(ur, kev)) * kevm[None, :]
        se = np.sin(w * np.outer(ur, kev)) * kevm[None, :]
        co = np.cos(w * np.outer(ur, ko))
        so = np.sin(w * np.outer(ur, ko))
        blocks.append(np.concatenate([ce, se, co, so], axis=1))
    c["tabs"] = np.concatenate(blocks, axis=1).astype(np.float32).astype(BF)

    ce5 = np.cos(w * 512.0 * kev) * kevm
    so5 = np.sin(w * 512.0 * ko)
    c["tab512"] = np.concatenate([ce5, so5]).astype(np.float32).astype(BF)

    # batch indicator for the cnt matmul: ind2[s, i*128+p] = (s//8 == i)
    s_batch = np.arange(SPC) // N
    cols = [np.tile((s_batch == i).astype(np.float32)[:, None], (1, 128))
            for i in range(BPC)]
    c["ind2"] = np.concatenate(cols, axis=1).astype(BF)        # [16, 256]
    return c


# ------------------------------------------------------------------- program
def _build_nc():
    nc = bacc.Bacc("TRN2", target_bir_lowering=False, debug=False,
                   num_devices=NCORES)

    def din(name, shape, dt):
        return nc.dram_tensor(name, shape, dt, kind="ExternalInput").ap()

    MW = 2 * SPC + FEW + 512                      # misc row: xq5 | tab512
    xq = din("xq", [128, 4 * UCH * SPC], BF16)    # butterfly ops a|b|c|d
    misc = din("misc", [1, MW], BF16)
    tabs = din("tabs", [128, UCH * CHW], BF16)    # chunk-major DFT tables
    ltf = din("ltf", [BPC, KTOT, T], BF16)        # onehot28 + 3 conv shifts
    w24 = din("w24", [KCONV, D], BF16)
    r4odd = din("r4odd", [KHOT, 2 * D], F32)
    ind2 = din("ind2", [SPC, BPC * 128], BF16)
    postab = din("postab", [128, NT * D], BF16)
    out = nc.dram_tensor("out", [BPC, 128, NT * D], BF16,
                         kind="ExternalOutput").ap()

    with tile.TileContext(nc) as tc:
        with (
            tc.tile_pool(name="consts", bufs=1) as cpool,
            tc.tile_pool(name="fwork", bufs=1) as fpool,
            tc.tile_pool(name="fpsum", bufs=1, space="PSUM") as fpsum,
            tc.tile_pool(name="mpsum", bufs=3, space="PSUM") as mpsum,
            tc.tile_pool(name="batch", bufs=1) as bpool,
            tc.tile_pool(name="outp", bufs=2) as opool,
            tc.tile_pool(name="cppool", bufs=2) as cppool,
        ):
            UW = UCH * SPC  # 64 butterfly cols

            # -------- SBUF tiles
            xq_sb = fpool.tile([128, 4 * UW], BF16, tag="xq")
            misc_sb = fpool.tile([1, MW], BF16, tag="misc")
            tabs_sb = cpool.tile([128, UCH * CHW], BF16, tag="tabs")
            ind2_sb = cpool.tile([SPC, BPC * 128], BF16, tag="ind2")
            r4odd_sb = cpool.tile([KHOT, 2 * D], F32, tag="r4odd")
            postab_sb = cpool.tile([128, NT * D], BF16, tag="postab")
            lts = [bpool.tile([KTOT, T], BF16, tag=f"lt{i}", name=f"lt{i}")
                   for i in range(BPC)]
            rhss = [bpool.tile([KTOT, D], BF16, tag=f"rhs{i}", name=f"rhs{i}")
                    for i in range(BPC)]
            xq5_sb = misc_sb[:, 0:2 * SPC]
            t512_sb = misc_sb[:, 2 * SPC:]

            # -------- single scalar HWDGE ring in strict priority order
            # (input reads share ~310 GB/s across all queues, so one
            # ordered ring beats parallel rings)
            nc.scalar.dma_start(xq_sb[:], xq)
            nc.scalar.dma_start(misc_sb[:], misc)
            for ch in range(UCH):
                nc.scalar.dma_start(tabs_sb[:, ch * CHW:(ch + 1) * CHW],
                                    tabs[:, ch * CHW:(ch + 1) * CHW])
            nc.scalar.dma_start(lts[0][:], ltf[0])
            nc.scalar.dma_start(lts[1][:], ltf[1])
            nc.scalar.dma_start(ind2_sb[:], ind2)
            nc.scalar.dma_start(r4odd_sb[:], r4odd)
            nc.scalar.dma_start(rhss[0][KHOT:KTOT, :], w24)
            nc.scalar.dma_start(rhss[1][KHOT:KTOT, :], w24)
            for q in range(4):
                nc.scalar.dma_start(
                    postab_sb[:, q * 4 * D:(q + 1) * 4 * D],
                    postab[:, q * 4 * D:(q + 1) * 4 * D])

            # -------- FFT phase: butterflies (vector)
            xa_sb = xq_sb[:, 0 * UW:1 * UW]
            xb_sb = xq_sb[:, 1 * UW:2 * UW]
            xc_sb = xq_sb[:, 2 * UW:3 * UW]
            xd_sb = xq_sb[:, 3 * UW:4 * UW]
            ab = fpool.tile([128, UW], BF16, tag="ab")
            nc.vector.tensor_add(ab[:], xa_sb, xb_sb)
            amb = fpool.tile([128, UW], BF16, tag="amb")
            nc.vector.tensor_sub(amb[:], xa_sb, xb_sb)
            cd = fpool.tile([128, UW], BF16, tag="cd")
            nc.vector.tensor_add(cd[:], xc_sb, xd_sb)
            cmd = fpool.tile([128, UW], BF16, tag="cmd")
            nc.vector.tensor_sub(cmd[:], xc_sb, xd_sb)
            pce = fpool.tile([128, UW], BF16, tag="pce")
            nc.vector.tensor_add(pce[:], ab[:], cd[:])
            pco = fpool.tile([128, UW], BF16, tag="pco")
            nc.vector.tensor_sub(pco[:], ab[:], cd[:])
            pse = fpool.tile([128, UW], BF16, tag="pse")
            nc.vector.tensor_sub(pse[:], amb[:], cmd[:])
            pso = fpool.tile([128, UW], BF16, tag="pso")
            nc.vector.tensor_add(pso[:], amb[:], cmd[:])

            # -------- FFT matmuls: psum bank A rows re_e/re_o/im_e/im_o at
            # bases 0/32/64/96; Nyquist+pad tail in ps_tail
            ps_main = fpsum.tile([112, 512], F32, tag="psmain")
            aux = fpsum.tile([128, 8], F32, tag="aux")
            opnds = {"ce": pce, "co": pco, "se": pse, "so": pso}
            off = {"ce": 0, "se": FEW, "co": 2 * FEW, "so": 2 * FEW + 512}
            for ch in range(UCH):
                st = (ch == 0)
                base = ch * CHW
                for nm, tb in (("ce", 0), ("se", 32)):
                    nc.tensor.matmul(aux[tb:tb + SPC, 0:4],
                                     opnds[nm][:, ch * SPC:(ch + 1) * SPC],
                                     tabs_sb[:, base + off[nm]:base + off[nm] + 4],
                                     start=st, stop=(nm == "se" and ch == UCH - 1),
                                     tile_position=(0, tb),
                                     skip_group_check=True)
                for pb, nm in ((0, "ce"), (32, "co"), (64, "se"), (96, "so")):
                    lhs = opnds[nm][:, ch * SPC:(ch + 1) * SPC]
                    if nm in ("ce", "se"):
                        cols = tabs_sb[:, base + off[nm] + 4:base + off[nm] + FEW]
                    else:
                        cols = tabs_sb[:, base + off[nm]:base + off[nm] + 512]
                    sp = (ch == UCH - 1) and pb in (32, 64)
                    nc.tensor.matmul(ps_main[pb:pb + SPC, :], lhs, cols,
                                     start=st, stop=sp,
                                     tile_position=(0, pb),
                                     skip_group_check=True)
            # u=512 residual: re_e += pce512*cos(pi k/2), im_o += pso512*sin
            nc.tensor.matmul(aux[0:SPC, 0:4], xq5_sb[:, 0:SPC],
                             t512_sb[:, 0:4],
                             start=False, stop=True, tile_position=(0, 0),
                             skip_group_check=True)
            nc.tensor.matmul(ps_main[0:SPC, :], xq5_sb[:, 0:SPC],
                             t512_sb[:, 4:FEW],
                             start=False, stop=True, tile_position=(0, 0),
                             skip_group_check=True)
            nc.tensor.matmul(ps_main[96:96 + SPC, :], xq5_sb[:, SPC:2 * SPC],
                             t512_sb[:, FEW:FEW + 512],
                             start=False, stop=True, tile_position=(0, 96),
                             skip_group_check=True)

            # -------- |X|^2: one big ACT square over all 4 DFT groups,
            # then quarter-aligned DVE adds (re^2 + im^2)
            mag_e = fpool.tile([SPC, 512], F32, tag="mag_e")
            mag_o = fpool.tile([SPC, 512], F32, tag="mag_o")
            mag_t = fpool.tile([SPC, 4], F32, tag="mag_t")
            sqa = fpool.tile([SPC, 512], F32, tag="sqa")
            sqb = fpool.tile([SPC, 512], F32, tag="sqb")
            sqc = fpool.tile([SPC, 4], F32, tag="sqc")
            nc.scalar.square(mag_e[:], ps_main[0:SPC, :])
            nc.scalar.square(sqa[:], ps_main[64:64 + SPC, :])
            nc.vector.tensor_add(mag_e[:], mag_e[:], sqa[:])
            nc.scalar.square(mag_o[:], ps_main[32:32 + SPC, :])
            nc.scalar.square(sqb[:], ps_main[96:96 + SPC, :])
            nc.vector.tensor_add(mag_o[:], mag_o[:], sqb[:])
            nc.scalar.square(mag_t[:], aux[0:SPC, 0:4])
            nc.scalar.square(sqc[:], aux[32:32 + SPC, 0:4])
            nc.vector.tensor_add(mag_t[:], mag_t[:], sqc[:])

            # strict >: Nyquist wins only if greater than every earlier bin
            lm_e = fpool.tile([SPC, 1], F32, tag="lm_e")
            nc.vector.reduce_max(lm_e[:], mag_e[:], axis=mybir.AxisListType.X)
            lm_o = fpool.tile([SPC, 1], F32, tag="lm_o")
            nc.vector.reduce_max(lm_o[:], mag_o[:], axis=mybir.AxisListType.X)
            lm_t = fpool.tile([SPC, 1], F32, tag="lm_t")
            nc.vector.reduce_max(lm_t[:], mag_t[:, 1:4],
                                 axis=mybir.AxisListType.X)
            lm2 = fpool.tile([SPC, 1], F32, tag="lm2")
            nc.vector.tensor_max(lm2[:], lm_e[:], lm_o[:])
            lmax = fpool.tile([SPC, 1], F32, tag="lmax")
            nc.vector.tensor_max(lmax[:], lm2[:], lm_t[:])
            isn = fpool.tile([SPC, 1], BF16, tag="isn")
            nc.vector.tensor_tensor(isn[:], mag_t[:, 0:1], lmax[:],
                                    op=mybir.AluOpType.is_gt)

            a_vecs, bq_vecs = [], []
            for i in range(BPC):
                ps_cnt = aux[:, 4 + i:5 + i]
                nc.tensor.matmul(ps_cnt, ind2_sb[:, i * 128:(i + 1) * 128],
                                 isn[:], start=True, stop=True,
                                 skip_group_check=True)
                a_vec = fpool.tile([128, 1], F32, tag=f"avec{i}")
                nc.vector.tensor_scalar(a_vec[:], ps_cnt, -0.125, 1.0,
                                        op0=mybir.AluOpType.mult,
                                        op1=mybir.AluOpType.add)
                bq_vec = fpool.tile([128, 1], F32, tag=f"bqvec{i}")
                nc.vector.tensor_scalar(bq_vec[:], ps_cnt, 1.0 / 32.0,
                                        None, op0=mybir.AluOpType.mult)
                a_vecs.append(a_vec)
                bq_vecs.append(bq_vec)

            # rhs: hot rows R4 + (cnt/32)*odd; conv rows copied from w24
            for i in range(BPC):
                nc.vector.scalar_tensor_tensor(
                    rhss[i][0:KHOT, :], r4odd_sb[:, D:2 * D],
                    bq_vecs[i][0:KHOT, :], r4odd_sb[:, 0:D],
                    op0=mybir.AluOpType.mult, op1=mybir.AluOpType.add)

            # prescaled a_i*postab for the copy+add drain lanes
            # (only tiles 6..15 of each batch use it)
            aposts = []
            for i in range(BPC):
                ap_t = cpool.tile([128, 10 * D], BF16, tag=f"apost{i}",
                                  name=f"apost{i}")
                nc.vector.tensor_scalar(
                    ap_t[:, 0:5 * D], postab_sb[:, 6 * D:11 * D],
                    a_vecs[i][:], None, op0=mybir.AluOpType.mult)
                nc.vector.tensor_scalar(
                    ap_t[:, 5 * D:10 * D], postab_sb[:, 11 * D:16 * D],
                    a_vecs[i][:], None, op0=mybir.AluOpType.mult)
                aposts.append(ap_t)

            # -------- main matmuls (pairs share a 2-bank psum tile) + drain
            # batches interleaved so both drain lanes run concurrently
            for g in range(NT // 4):
                for i in range(BPC):
                    ot4 = opool.tile([128, 4 * D], BF16, tag=f"ot{i}",
                                     name=f"ot{i}")
                    for h in range(2):
                        ps2 = mpsum.tile([128, 2 * D], F32, tag="ps",
                                         name="ps")
                        for s in range(2):
                            ti = g * 4 + h * 2 + s
                            nc.tensor.matmul(
                                ps2[:, s * D:(s + 1) * D],
                                lts[i][:, ti * 128:(ti + 1) * 128],
                                rhss[i][:], start=True, stop=True)
                        pr = g * 2 + h      # pair index 0..7 within batch
                        tlo = (g * 4 + h * 2) * D
                        if pr < 3:
                            # lane A: fused DVE drain straight from PSUM
                            nc.vector.scalar_tensor_tensor(
                                ot4[:, h * 2 * D:(h + 1) * 2 * D],
                                postab_sb[:, tlo:tlo + 2 * D], a_vecs[i][:],
                                ps2[:], op0=mybir.AluOpType.mult,
                                op1=mybir.AluOpType.add)
                        else:
                            # lanes B/C: ACT drains PSUM->bf16, add on
                            # gpsimd (pairs 3-5) or vector (pairs 6-7)
                            cp2 = cppool.tile([128, 2 * D], BF16, tag="cp",
                                              name="cp")
                            nc.scalar.copy(cp2[:], ps2[:])
                            alo = (pr - 3) * 2 * D
                            eng = nc.gpsimd if pr < 5 else nc.vector
                            eng.tensor_add(
                                ot4[:, h * 2 * D:(h + 1) * 2 * D],
                                aposts[i][:, alo:alo + 2 * D], cp2[:])
                    nc.sync.dma_start(
                        out[i, :, g * 4 * D:(g + 1) * 4 * D], ot4[:])
    nc.compile()
    return nc


def _get_nc():
    if "nc" not in _cache:
        _cache["nc"] = _build_nc()
    return _cache["nc"]


def _host_inputs(x, x_mark, conv_w):
    # lt rows: 0:28 onehot (hot[b,j,t] = x_mark[b,t,j//7] == j%7),
    #          28:52 three circular shifts of x^T
    xm = np.asarray(x_mark).astype(np.int64)               # [16, 2048, 4]
    j = np.arange(KHOT)
    hot = (xm[:, :, j // 7] == (j % 7)[None, None, :])     # [16, 2048, 28]
    hot = hot.transpose(0, 2, 1).astype(np.float32)        # [16, 28, 2048]
    xt = np.ascontiguousarray(x.transpose(0, 2, 1))        # [16, 8, 2048]
    xtp = np.concatenate([xt[:, :, -1:], xt, xt[:, :, :1]], axis=2)
    ltf = np.concatenate(
        [hot] + [xtp[:, :, k:k + T] for k in range(3)], axis=1)  # [16,52,T]
    ltf = np.ascontiguousarray(ltf).astype(BF)
    # per-core butterfly operands [tt, ch*16 + s], u = ch*128+tt (0..511)
    uu = np.arange(UCH * 128)
    mid = uu >= 1
    ai = uu
    bi = np.where(mid, (T - uu) % T, 0)
    ci = np.where(mid, NYQ - uu, NYQ)
    di = np.where(mid, NYQ + uu, 0)
    dm = mid
    quads = []
    x5 = []
    for core in range(NCORES):
        xs = x[core * BPC:(core + 1) * BPC]                # [2, 2048, 8]
        xflat = xs.transpose(1, 0, 2).reshape(T, SPC)      # [t, s]
        qs = []
        for idx, msk in ((ai, None), (bi, None), (ci, None), (di, dm)):
            arr = xflat[idx]
            if msk is not None:
                arr = arr * msk[:, None]
            qs.append(np.ascontiguousarray(
                arr.reshape(UCH, 128, SPC).transpose(1, 0, 2)
                   .reshape(128, UCH * SPC)).astype(BF))
        quads.append(np.concatenate(qs, axis=1))           # [128, 4*64]
        pce5 = xflat[512] + xflat[1536]
        pso5 = xflat[512] - xflat[1536]
        x5.append(np.concatenate([pce5, pso5]).astype(BF))
    # conv weight rows (k, n): w24[k*8+n, d] = conv_w[d, n, k]
    w24 = np.ascontiguousarray(
        conv_w.transpose(2, 1, 0).reshape(KCONV, D)).astype(BF)
    return ltf, quads, x5, w24


def make_in_maps(x, x_mark, conv_w):
    if "consts" not in _cache:
        _cache["consts"] = _host_constants()
    c = _cache["consts"]
    ltf, quads, x5, w24 = _host_inputs(x, x_mark, conv_w)
    in_maps = []
    for core in range(NCORES):
        b0 = core * BPC
        misc = np.concatenate([x5[core], c["tab512"]])[None, :]
        in_maps.append({
            "xq": quads[core],
            "misc": np.ascontiguousarray(misc),
            "tabs": c["tabs"],
            "ltf": np.ascontiguousarray(ltf[b0:b0 + BPC]),
            "w24": w24,
            "r4odd": c["r4odd"],
            "ind2": c["ind2"],
            "postab": c["postab"],
        })
    return in_maps


# -------------------------------------------------------------------- driver
def kernel(**inputs):
    x = np.asarray(inputs["x"], dtype=np.float32)          # [16, 2048, 8]
    x_mark = np.asarray(inputs["x_mark"])                  # [16, 2048, 4] int
    conv_w = np.asarray(inputs["conv_w"], dtype=np.float32)  # [512, 8, 3]

    in_maps = make_in_maps(x, x_mark, conv_w)
    nc = _get_nc()
    kw = {}
    if TRACE:
        kw = dict(trace=True, tmpdir=TRACE_DIR)
    br = run_bass_kernel_spmd(nc, in_maps, list(range(NCORES)), **kw)
    if TRACE:
        _cache["last_results"] = br

    outp = np.empty((B, T, D), dtype=np.float32)
    for core in range(NCORES):
        o = np.asarray(br.results[core]["out"]).astype(np.float32)
        o = o.reshape(BPC, 128, NT, D).transpose(0, 2, 1, 3).reshape(BPC, T, D)
        outp[core * BPC:(core + 1) * BPC] = o
    return outp


# revision 18
# speedup vs baseline: 1.1292x; 1.0892x over previous
"""Trainium2 Bass kernel for nn_DataEmbedding_cycle_pos.

Math (B=16, T=2048, N=8, D=512), out[b,t,:] =
    conv(x)               Conv1d(N->D, k=3, circular)        -> matmul K=24
  + temporal(x_mark)      sum of 4 fixed-table lookups; host precomputes the
                          28-row onehot so it's onehot28 @ R4  -> matmul K=28
  + cycle-positional      periods = clip(T/freq[argmax |rfft|], 1, T); for
                          T=2048 the period is 2048 unless the argmax is
                          exactly the Nyquist bin (then 1.0).  Per (b,n) only
                          the bit "is Nyquist the strict max" matters:
                            cyc[b] = (1-cnt/8)*postab + (cnt/8)*row01
                          cnt = #Nyquist-max series in batch b.
  The row01 (odd-column ones) term folds into the onehot matmul rows since
  sum(onehot) == 4 exactly:  R4 + (cnt/32)*odd.

Sharding: batch-parallel (2 batches/core).  The |rfft|^2 argmax test is
computed per core for its OWN 16 series (no collectives) via a
quarter-size DFT: double time-fold (u = 0..511 in 4 row-chunks, plus a
K=1 residual matmul for u=512) x frequency-parity split, in bf16 matmuls.

Engine/DMA layout (each dma_start costs ~700ns of issuing-engine time, so
inputs are coalesced host-side into 7 transfers):
  - gpsimd SWDGE (436 GB/s ring): the two big tables (tabs, postab)
  - scalar HWDGE: the 7 coalesced small inputs, ACT squares, batch-1
    PSUM->SBUF drains (pairs of tiles, [128,1024])
  - vector: FFT glue, batch-0 fused drain (STT from PSUM), batch-1
    prescale a1*postab and bf16 adds (2x mode)
  - sync HWDGE: 8 output DMAs ([BPC,128,NT*D] bf16 layout, 4KB packets)
"""
import sys, os

sys.path.insert(0, "/opt/trn_rl_repo")
import numpy as np
import ml_dtypes

import concourse.bass as bass
import concourse.bacc as bacc
import concourse.mybir as mybir
import concourse.tile as tile
from concourse.bass_utils import run_bass_kernel_spmd

B, T, N, D = 16, 2048, 8, 512
NCORES = 8
BPC = B // NCORES          # batches per core
SPC = BPC * N              # series per core (16)
NT = T // 128              # 128-row time tiles per batch
KCONV = 3 * N              # 24 conv rows
KHOT = 28                  # 4 features x 7 index values
KTOT = KCONV + KHOT        # 52
NYQ = T // 2               # 1024
UCH = 4                    # u chunks of 128 covering u=0..511
FEW = 516                  # even-parity freq cols (Nyquist first, 3 pad)
CHW = 2 * FEW + 2 * 512    # 2056 table cols per chunk: ce|se|co|so

F32 = mybir.dt.float32
BF16 = mybir.dt.bfloat16
BF = ml_dtypes.bfloat16

TRACE = False
TRACE_DIR = None

_cache = {}


# ----------------------------------------------------------------- constants
def _div_term():
    # mirror reference: exp(arange(0,512,2) * (-ln 10000 / 512)) in f32
    return np.exp(
        np.arange(0, D, 2, dtype=np.float32) * np.float32(-np.log(10000.0) / D)
    ).astype(np.float32)


def _fixed_rows(nrows):
    pos = np.arange(nrows, dtype=np.float32)[:, None]
    ang = (pos * _div_term()[None, :]).astype(np.float32)
    tab = np.zeros((nrows, D), dtype=np.float32)
    tab[:, 0::2] = np.sin(ang)
    tab[:, 1::2] = np.cos(ang)
    return tab


def _host_constants():
    c = {}
    postab = _fixed_rows(T)  # [2048, 512]
    # SBUF layout [128(tt), 16 tiles * 512]
    c["postab"] = np.ascontiguousarray(
        postab.reshape(NT, 128, D).transpose(1, 0, 2).reshape(128, NT * D)
    ).astype(BF)
    r7 = _fixed_rows(7)
    odd = np.zeros((D,), dtype=np.float32)
    odd[1::2] = 1.0
    r4 = np.tile(r7, (4, 1)).astype(np.float32)
    odd28 = np.tile(odd[None, :], (KHOT, 1)).astype(np.float32)
    c["r4odd"] = np.ascontiguousarray(
        np.concatenate([r4, odd28], axis=1))           # [28, 1024] f32

    # quarter DFT tables (double time-fold, frequencies split by parity):
    # chunk-major packed [128(tt), 4 ch * (ce|se|co|so)] covering u=0..511;
    # the u=512 row is a separate residual (ce | so only; sin(pi*k/2)=0 for
    # even k and cos(pi*k/2)=0 for odd k kill se/co), packed into `misc`.
    w = 2.0 * np.pi / T
    ke = np.arange(0, NYQ + 1, 2, dtype=np.float64)            # 513 even
    ko = np.arange(1, NYQ, 2, dtype=np.float64)                # 512 odd
    kep = np.concatenate([ke[512:], ke[:512]])                 # nyq first
    kev = np.zeros(FEW, dtype=np.float64); kev[:513] = kep
    kevm = np.zeros(FEW); kevm[:513] = 1.0

    uu = np.arange(UCH * 128, dtype=np.float64)                # u = 0..511
    blocks = []
    for ch in range(UCH):
        ur = uu[ch * 128:(ch + 1) * 128]
        ce = np.cos(w * np.outer(ur, kev)) * kevm[None, :]
        se = np.sin(w * np.outer(ur, kev)) * kevm[None, :]
        co = np.cos(w * np.outer(ur, ko))
        so = np.sin(w * np.outer(ur, ko))
        blocks.append(np.concatenate([ce, se, co, so], axis=1))
    c["tabs"] = np.concatenate(blocks, axis=1).astype(np.float32).astype(BF)

    ce5 = np.cos(w * 512.0 * kev) * kevm
    so5 = np.sin(w * 512.0 * ko)
    c["tab512"] = np.concatenate([ce5, so5]).astype(np.float32).astype(BF)

    # batch indicator for the cnt matmul: ind2[s, i*128+p] = (s//8 == i)
    s_batch = np.arange(SPC) // N
    cols = [np.tile((s_batch == i).astype(np.float32)[:, None], (1, 128))
            for i in range(BPC)]
    c["ind2"] = np.concatenate(cols, axis=1).astype(BF)        # [16, 256]
    return c


# ------------------------------------------------------------------- program
def _build_nc():
    nc = bacc.Bacc("TRN2", target_bir_lowering=False, debug=False,
                   num_devices=NCORES)

    def din(name, shape, dt):
        return nc.dram_tensor(name, shape, dt, kind="ExternalInput").ap()

    MW = 2 * SPC + FEW + 512                      # misc row: xq5 | tab512
    xq = din("xq", [128, 4 * UCH * SPC], BF16)    # butterfly ops a|b|c|d
    misc = din("misc", [1, MW], BF16)
    tabs = din("tabs", [128, UCH * CHW], BF16)    # chunk-major DFT tables
    ltf = din("ltf", [BPC, KTOT, T], BF16)        # onehot28 + 3 conv shifts
    w24 = din("w24", [KCONV, D], BF16)
    r4odd = din("r4odd", [KHOT, 2 * D], F32)
    ind2 = din("ind2", [SPC, BPC * 128], BF16)
    postab = din("postab", [128, NT * D], BF16)
    out = nc.dram_tensor("out", [BPC, 128, NT * D], BF16,
                         kind="ExternalOutput").ap()

    with tile.TileContext(nc) as tc:
        with (
            tc.tile_pool(name="consts", bufs=1) as cpool,
            tc.tile_pool(name="fwork", bufs=1) as fpool,
            tc.tile_pool(name="fpsum", bufs=1, space="PSUM") as fpsum,
            tc.tile_pool(name="mpsum", bufs=3, space="PSUM") as mpsum,
            tc.tile_pool(name="batch", bufs=1) as bpool,
            tc.tile_pool(name="outp", bufs=3) as opool,
            tc.tile_pool(name="cppool", bufs=4) as cppool,
        ):
            UW = UCH * SPC  # 64 butterfly cols

            # -------- SBUF tiles
            xq_sb = fpool.tile([128, 4 * UW], BF16, tag="xq")
            misc_sb = fpool.tile([1, MW], BF16, tag="misc")
            tabs_sb = cpool.tile([128, UCH * CHW], BF16, tag="tabs")
            ind2_sb = cpool.tile([SPC, BPC * 128], BF16, tag="ind2")
            r4odd_sb = cpool.tile([KHOT, 2 * D], F32, tag="r4odd")
            postab_sb = cpool.tile([128, NT * D], BF16, tag="postab")
            lts = [bpool.tile([KTOT, T], BF16, tag=f"lt{i}", name=f"lt{i}")
                   for i in range(BPC)]
            rhss = [bpool.tile([KTOT, D], BF16, tag=f"rhs{i}", name=f"rhs{i}")
                    for i in range(BPC)]
            xq5_sb = misc_sb[:, 0:2 * SPC]
            t512_sb = misc_sb[:, 2 * SPC:]

            # -------- single scalar HWDGE ring in strict priority order
            # (input reads share ~310 GB/s across all queues, so one
            # ordered ring beats parallel rings)
            nc.scalar.dma_start(xq_sb[:], xq)
            nc.scalar.dma_start(misc_sb[:], misc)
            for ch in range(UCH):
                nc.scalar.dma_start(tabs_sb[:, ch * CHW:(ch + 1) * CHW],
                                    tabs[:, ch * CHW:(ch + 1) * CHW])
            nc.scalar.dma_start(lts[0][:], ltf[0])
            nc.scalar.dma_start(lts[1][:], ltf[1])
            nc.scalar.dma_start(ind2_sb[:], ind2)
            nc.scalar.dma_start(r4odd_sb[:], r4odd)
            nc.scalar.dma_start(rhss[0][KHOT:KTOT, :], w24)
            nc.scalar.dma_start(rhss[1][KHOT:KTOT, :], w24)
            for q in range(4):
                nc.scalar.dma_start(
                    postab_sb[:, q * 4 * D:(q + 1) * 4 * D],
                    postab[:, q * 4 * D:(q + 1) * 4 * D])

            # -------- FFT phase: butterflies (vector)
            xa_sb = xq_sb[:, 0 * UW:1 * UW]
            xb_sb = xq_sb[:, 1 * UW:2 * UW]
            xc_sb = xq_sb[:, 2 * UW:3 * UW]
            xd_sb = xq_sb[:, 3 * UW:4 * UW]
            ab = fpool.tile([128, UW], BF16, tag="ab")
            nc.vector.tensor_add(ab[:], xa_sb, xb_sb)
            amb = fpool.tile([128, UW], BF16, tag="amb")
            nc.vector.tensor_sub(amb[:], xa_sb, xb_sb)
            cd = fpool.tile([128, UW], BF16, tag="cd")
            nc.vector.tensor_add(cd[:], xc_sb, xd_sb)
            cmd = fpool.tile([128, UW], BF16, tag="cmd")
            nc.vector.tensor_sub(cmd[:], xc_sb, xd_sb)
            pce = fpool.tile([128, UW], BF16, tag="pce")
            nc.vector.tensor_add(pce[:], ab[:], cd[:])
            pco = fpool.tile([128, UW], BF16, tag="pco")
            nc.vector.tensor_sub(pco[:], ab[:], cd[:])
            pse = fpool.tile([128, UW], BF16, tag="pse")
            nc.vector.tensor_sub(pse[:], amb[:], cmd[:])
            pso = fpool.tile([128, UW], BF16, tag="pso")
            nc.vector.tensor_add(pso[:], amb[:], cmd[:])

            # -------- FFT matmuls: psum bank A rows re_e/re_o/im_e/im_o at
            # bases 0/32/64/96; Nyquist+pad tail in ps_tail
            ps_main = fpsum.tile([112, 512], F32, tag="psmain")
            aux = fpsum.tile([128, 8], F32, tag="aux")
            opnds = {"ce": pce, "co": pco, "se": pse, "so": pso}
            off = {"ce": 0, "se": FEW, "co": 2 * FEW, "so": 2 * FEW + 512}
            for ch in range(UCH):
                st = (ch == 0)
                base = ch * CHW
                for nm, tb in (("ce", 0), ("se", 32)):
                    nc.tensor.matmul(aux[tb:tb + SPC, 0:4],
                                     opnds[nm][:, ch * SPC:(ch + 1) * SPC],
                                     tabs_sb[:, base + off[nm]:base + off[nm] + 4],
                                     start=st, stop=(nm == "se" and ch == UCH - 1),
                                     tile_position=(0, tb),
                                     skip_group_check=True)
                for pb, nm in ((0, "ce"), (32, "co"), (64, "se"), (96, "so")):
                    lhs = opnds[nm][:, ch * SPC:(ch + 1) * SPC]
                    if nm in ("ce", "se"):
                        cols = tabs_sb[:, base + off[nm] + 4:base + off[nm] + FEW]
                    else:
                        cols = tabs_sb[:, base + off[nm]:base + off[nm] + 512]
                    sp = (ch == UCH - 1) and pb in (32, 64)
                    nc.tensor.matmul(ps_main[pb:pb + SPC, :], lhs, cols,
                                     start=st, stop=sp,
                                     tile_position=(0, pb),
                                     skip_group_check=True)
            # u=512 residual: re_e += pce512*cos(pi k/2), im_o += pso512*sin
            nc.tensor.matmul(aux[0:SPC, 0:4], xq5_sb[:, 0:SPC],
                             t512_sb[:, 0:4],
                             start=False, stop=True, tile_position=(0, 0),
                             skip_group_check=True)
            nc.tensor.matmul(ps_main[0:SPC, :], xq5_sb[:, 0:SPC],
                             t512_sb[:, 4:FEW],
                             start=False, stop=True, tile_position=(0, 0),
                             skip_group_check=True)
            nc.tensor.matmul(ps_main[96:96 + SPC, :], xq5_sb[:, SPC:2 * SPC],
                             t512_sb[:, FEW:FEW + 512],
                             start=False, stop=True, tile_position=(0, 96),
                             skip_group_check=True)

            # -------- |X|^2: one big ACT square over all 4 DFT groups,
            # then quarter-aligned DVE adds (re^2 + im^2)
            mag_e = fpool.tile([SPC, 512], F32, tag="mag_e")
            mag_o = fpool.tile([SPC, 512], F32, tag="mag_o")
            mag_t = fpool.tile([SPC, 4], F32, tag="mag_t")
            sqa = fpool.tile([SPC, 512], F32, tag="sqa")
            sqb = fpool.tile([SPC, 512], F32, tag="sqb")
            sqc = fpool.tile([SPC, 4], F32, tag="sqc")
            nc.scalar.square(mag_e[:], ps_main[0:SPC, :])
            nc.scalar.square(sqa[:], ps_main[64:64 + SPC, :])
            nc.vector.tensor_add(mag_e[:], mag_e[:], sqa[:])
            nc.scalar.square(mag_o[:], ps_main[32:32 + SPC, :])
            nc.scalar.square(sqb[:], ps_main[96:96 + SPC, :])
            nc.vector.tensor_add(mag_o[:], mag_o[:], sqb[:])
            nc.scalar.square(mag_t[:], aux[0:SPC, 0:4])
            nc.scalar.square(sqc[:], aux[32:32 + SPC, 0:4])
            nc.vector.tensor_add(mag_t[:], mag_t[:], sqc[:])

            # strict >: Nyquist wins only if greater than every earlier bin
            lm_e = fpool.tile([SPC, 1], F32, tag="lm_e")
            nc.vector.reduce_max(lm_e[:], mag_e[:], axis=mybir.AxisListType.X)
            lm_o = fpool.tile([SPC, 1], F32, tag="lm_o")
            nc.vector.reduce_max(lm_o[:], mag_o[:], axis=mybir.AxisListType.X)
            lm_t = fpool.tile([SPC, 1], F32, tag="lm_t")
            nc.vector.reduce_max(lm_t[:], mag_t[:, 1:4],
                                 axis=mybir.AxisListType.X)
            lm2 = fpool.tile([SPC, 1], F32, tag="lm2")
            nc.vector.tensor_max(lm2[:], lm_e[:], lm_o[:])
            lmax = fpool.tile([SPC, 1], F32, tag="lmax")
            nc.vector.tensor_max(lmax[:], lm2[:], lm_t[:])
            isn = fpool.tile([SPC, 1], BF16, tag="isn")
            nc.vector.tensor_tensor(isn[:], mag_t[:, 0:1], lmax[:],
                                    op=mybir.AluOpType.is_gt)

            a_vecs, bq_vecs = [], []
            for i in range(BPC):
                ps_cnt = aux[:, 4 + i:5 + i]
                nc.tensor.matmul(ps_cnt, ind2_sb[:, i * 128:(i + 1) * 128],
                                 isn[:], start=True, stop=True,
                                 skip_group_check=True)
                a_vec = fpool.tile([128, 1], F32, tag=f"avec{i}")
                nc.vector.tensor_scalar(a_vec[:], ps_cnt, -0.125, 1.0,
                                        op0=mybir.AluOpType.mult,
                                        op1=mybir.AluOpType.add)
                bq_vec = fpool.tile([128, 1], F32, tag=f"bqvec{i}")
                nc.vector.tensor_scalar(bq_vec[:], ps_cnt, 1.0 / 32.0,
                                        None, op0=mybir.AluOpType.mult)
                a_vecs.append(a_vec)
                bq_vecs.append(bq_vec)

            # rhs: hot rows R4 + (cnt/32)*odd; conv rows copied from w24
            for i in range(BPC):
                nc.vector.scalar_tensor_tensor(
                    rhss[i][0:KHOT, :], r4odd_sb[:, D:2 * D],
                    bq_vecs[i][0:KHOT, :], r4odd_sb[:, 0:D],
                    op0=mybir.AluOpType.mult, op1=mybir.AluOpType.add)

            # prescaled a_i*postab for the copy+add drain lanes
            # (only tiles 6..15 of each batch use it)
            aposts = []
            for i in range(BPC):
                ap_t = cpool.tile([128, 12 * D], BF16, tag=f"apost{i}",
                                  name=f"apost{i}")
                nc.vector.tensor_scalar(
                    ap_t[:, 0:6 * D], postab_sb[:, 4 * D:10 * D],
                    a_vecs[i][:], None, op0=mybir.AluOpType.mult)
                nc.vector.tensor_scalar(
                    ap_t[:, 6 * D:12 * D], postab_sb[:, 10 * D:16 * D],
                    a_vecs[i][:], None, op0=mybir.AluOpType.mult)
                aposts.append(ap_t)

            # -------- main matmuls (pairs share a 2-bank psum tile) + drain
            # batches interleaved so both drain lanes run concurrently
            for g in range(NT // 4):
                for i in range(BPC):
                    ot4 = opool.tile([128, 4 * D], BF16, tag=f"ot{i}",
                                     name=f"ot{i}")
                    for h in range(2):
                        ps2 = mpsum.tile([128, 2 * D], F32, tag="ps",
                                         name="ps")
                        for s in range(2):
                            ti = g * 4 + h * 2 + s
                            nc.tensor.matmul(
                                ps2[:, s * D:(s + 1) * D],
                                lts[i][:, ti * 128:(ti + 1) * 128],
                                rhss[i][:], start=True, stop=True)
                        pr = g * 2 + h      # pair index 0..7 within batch
                        tlo = (g * 4 + h * 2) * D
                        if pr < 2:
                            # lane A: fused DVE drain straight from PSUM
                            nc.vector.scalar_tensor_tensor(
                                ot4[:, h * 2 * D:(h + 1) * 2 * D],
                                postab_sb[:, tlo:tlo + 2 * D], a_vecs[i][:],
                                ps2[:], op0=mybir.AluOpType.mult,
                                op1=mybir.AluOpType.add)
                        else:
                            # lanes B/C: ACT drains PSUM->bf16, add on
                            # gpsimd (pairs 3-5) or vector (pairs 6-7)
                            cp2 = cppool.tile([128, 2 * D], BF16, tag="cp",
                                              name="cp")
                            nc.scalar.copy(cp2[:], ps2[:])
                            alo = (pr - 2) * 2 * D
                            nc.vector.tensor_add(
                                ot4[:, h * 2 * D:(h + 1) * 2 * D],
                                aposts[i][:, alo:alo + 2 * D], cp2[:])
                    nc.sync.dma_start(
                        out[i, :, g * 4 * D:(g + 1) * 4 * D], ot4[:])
    nc.compile()
    return nc


def _get_nc():
    if "nc" not in _cache:
        _cache["nc"] = _build_nc()
    return _cache["nc"]


def _host_inputs(x, x_mark, conv_w):
    # lt rows: 0:28 onehot (hot[b,j,t] = x_mark[b,t,j//7] == j%7),
    #          28:52 three circular shifts of x^T
    xm = np.asarray(x_mark).astype(np.int64)               # [16, 2048, 4]
    j = np.arange(KHOT)
    hot = (xm[:, :, j // 7] == (j % 7)[None, None, :])     # [16, 2048, 28]
    hot = hot.transpose(0, 2, 1).astype(np.float32)        # [16, 28, 2048]
    xt = np.ascontiguousarray(x.transpose(0, 2, 1))        # [16, 8, 2048]
    xtp = np.concatenate([xt[:, :, -1:], xt, xt[:, :, :1]], axis=2)
    ltf = np.concatenate(
        [hot] + [xtp[:, :, k:k + T] for k in range(3)], axis=1)  # [16,52,T]
    ltf = np.ascontiguousarray(ltf).astype(BF)
    # per-core butterfly operands [tt, ch*16 + s], u = ch*128+tt (0..511)
    uu = np.arange(UCH * 128)
    mid = uu >= 1
    ai = uu
    bi = np.where(mid, (T - uu) % T, 0)
    ci = np.where(mid, NYQ - uu, NYQ)
    di = np.where(mid, NYQ + uu, 0)
    dm = mid
    quads = []
    x5 = []
    for core in range(NCORES):
        xs = x[core * BPC:(core + 1) * BPC]                # [2, 2048, 8]
        xflat = xs.transpose(1, 0, 2).reshape(T, SPC)      # [t, s]
        qs = []
        for idx, msk in ((ai, None), (bi, None), (ci, None), (di, dm)):
            arr = xflat[idx]
            if msk is not None:
                arr = arr * msk[:, None]
            qs.append(np.ascontiguousarray(
                arr.reshape(UCH, 128, SPC).transpose(1, 0, 2)
                   .reshape(128, UCH * SPC)).astype(BF))
        quads.append(np.concatenate(qs, axis=1))           # [128, 4*64]
        pce5 = xflat[512] + xflat[1536]
        pso5 = xflat[512] - xflat[1536]
        x5.append(np.concatenate([pce5, pso5]).astype(BF))
    # conv weight rows (k, n): w24[k*8+n, d] = conv_w[d, n, k]
    w24 = np.ascontiguousarray(
        conv_w.transpose(2, 1, 0).reshape(KCONV, D)).astype(BF)
    return ltf, quads, x5, w24


def make_in_maps(x, x_mark, conv_w):
    if "consts" not in _cache:
        _cache["consts"] = _host_constants()
    c = _cache["consts"]
    ltf, quads, x5, w24 = _host_inputs(x, x_mark, conv_w)
    in_maps = []
    for core in range(NCORES):
        b0 = core * BPC
        misc = np.concatenate([x5[core], c["tab512"]])[None, :]
        in_maps.append({
            "xq": quads[core],
            "misc": np.ascontiguousarray(misc),
            "tabs": c["tabs"],
            "ltf": np.ascontiguousarray(ltf[b0:b0 + BPC]),
            "w24": w24,
            "r4odd": c["r4odd"],
            "ind2": c["ind2"],
            "postab": c["postab"],
        })
    return in_maps


# -------------------------------------------------------------------- driver
def kernel(**inputs):
    x = np.asarray(inputs["x"], dtype=np.float32)          # [16, 2048, 8]
    x_mark = np.asarray(inputs["x_mark"])                  # [16, 2048, 4] int
    conv_w = np.asarray(inputs["conv_w"], dtype=np.float32)  # [512, 8, 3]

    in_maps = make_in_maps(x, x_mark, conv_w)
    nc = _get_nc()
    kw = {}
    if TRACE:
        kw = dict(trace=True, tmpdir=TRACE_DIR)
    br = run_bass_kernel_spmd(nc, in_maps, list(range(NCORES)), **kw)
    if TRACE:
        _cache["last_results"] = br

    outp = np.empty((B, T, D), dtype=np.float32)
    for core in range(NCORES):
        o = np.asarray(br.results[core]["out"]).astype(np.float32)
        o = o.reshape(BPC, 128, NT, D).transpose(0, 2, 1, 3).reshape(BPC, T, D)
        outp[core * BPC:(core + 1) * BPC] = o
    return outp


# revision 19
# speedup vs baseline: 1.1316x; 1.0021x over previous
"""Trainium2 Bass kernel for nn_DataEmbedding_cycle_pos.

Math (B=16, T=2048, N=8, D=512), out[b,t,:] =
    conv(x)               Conv1d(N->D, k=3, circular)        -> matmul K=24
  + temporal(x_mark)      sum of 4 fixed-table lookups; host precomputes the
                          28-row onehot so it's onehot28 @ R4  -> matmul K=28
  + cycle-positional      periods = clip(T/freq[argmax |rfft|], 1, T); for
                          T=2048 the period is 2048 unless the argmax is
                          exactly the Nyquist bin (then 1.0).  Per (b,n) only
                          the bit "is Nyquist the strict max" matters:
                            cyc[b] = (1-cnt/8)*postab + (cnt/8)*row01
                          cnt = #Nyquist-max series in batch b.
  The row01 (odd-column ones) term folds into the onehot matmul rows since
  sum(onehot) == 4 exactly:  R4 + (cnt/32)*odd.

Sharding: batch-parallel (2 batches/core).  The |rfft|^2 argmax test is
computed per core for its OWN 16 series (no collectives) via a
quarter-size DFT: double time-fold (u = 0..511 in 4 row-chunks, plus a
K=1 residual matmul for u=512) x frequency-parity split, in bf16 matmuls.

Engine/DMA layout (each dma_start costs ~700ns of issuing-engine time, so
inputs are coalesced host-side into 7 transfers):
  - gpsimd SWDGE (436 GB/s ring): the two big tables (tabs, postab)
  - scalar HWDGE: the 7 coalesced small inputs, ACT squares, batch-1
    PSUM->SBUF drains (pairs of tiles, [128,1024])
  - vector: FFT glue, batch-0 fused drain (STT from PSUM), batch-1
    prescale a1*postab and bf16 adds (2x mode)
  - sync HWDGE: 8 output DMAs ([BPC,128,NT*D] bf16 layout, 4KB packets)
"""
import sys, os

sys.path.insert(0, "/opt/trn_rl_repo")
import numpy as np
import ml_dtypes

import concourse.bass as bass
import concourse.bacc as bacc
import concourse.mybir as mybir
import concourse.tile as tile
from concourse.bass_utils import run_bass_kernel_spmd

B, T, N, D = 16, 2048, 8, 512
NCORES = 8
BPC = B // NCORES          # batches per core
SPC = BPC * N              # series per core (16)
NT = T // 128              # 128-row time tiles per batch
KCONV = 3 * N              # 24 conv rows
KHOT = 28                  # 4 features x 7 index values
KTOT = KCONV + KHOT        # 52
NYQ = T // 2               # 1024
UCH = 4                    # u chunks of 128 covering u=0..511
FEW = 516                  # even-parity freq cols (Nyquist first, 3 pad)
CHW = 2 * FEW + 2 * 512    # 2056 table cols per chunk: ce|se|co|so

F32 = mybir.dt.float32
BF16 = mybir.dt.bfloat16
BF = ml_dtypes.bfloat16

TRACE = False
TRACE_DIR = None

_cache = {}


# ----------------------------------------------------------------- constants
def _div_term():
    # mirror reference: exp(arange(0,512,2) * (-ln 10000 / 512)) in f32
    return np.exp(
        np.arange(0, D, 2, dtype=np.float32) * np.float32(-np.log(10000.0) / D)
    ).astype(np.float32)


def _fixed_rows(nrows):
    pos = np.arange(nrows, dtype=np.float32)[:, None]
    ang = (pos * _div_term()[None, :]).astype(np.float32)
    tab = np.zeros((nrows, D), dtype=np.float32)
    tab[:, 0::2] = np.sin(ang)
    tab[:, 1::2] = np.cos(ang)
    return tab


def _host_constants():
    c = {}
    postab = _fixed_rows(T)  # [2048, 512]
    # SBUF layout [128(tt), 16 tiles * 512]
    c["postab"] = np.ascontiguousarray(
        postab.reshape(NT, 128, D).transpose(1, 0, 2).reshape(128, NT * D)
    ).astype(BF)
    r7 = _fixed_rows(7)
    odd = np.zeros((D,), dtype=np.float32)
    odd[1::2] = 1.0
    r4 = np.tile(r7, (4, 1)).astype(np.float32)
    odd28 = np.tile(odd[None, :], (KHOT, 1)).astype(np.float32)
    c["r4odd"] = np.ascontiguousarray(
        np.concatenate([r4, odd28], axis=1))           # [28, 1024] f32

    # quarter DFT tables (double time-fold, frequencies split by parity):
    # chunk-major packed [128(tt), 4 ch * (ce|se|co|so)] covering u=0..511;
    # the u=512 row is a separate residual (ce | so only; sin(pi*k/2)=0 for
    # even k and cos(pi*k/2)=0 for odd k kill se/co), packed into `misc`.
    w = 2.0 * np.pi / T
    ke = np.arange(0, NYQ + 1, 2, dtype=np.float64)            # 513 even
    ko = np.arange(1, NYQ, 2, dtype=np.float64)                # 512 odd
    kep = np.concatenate([ke[512:], ke[:512]])                 # nyq first
    kev = np.zeros(FEW, dtype=np.float64); kev[:513] = kep
    kevm = np.zeros(FEW); kevm[:513] = 1.0

    uu = np.arange(UCH * 128, dtype=np.float64)                # u = 0..511
    blocks = []
    for ch in range(UCH):
        ur = uu[ch * 128:(ch + 1) * 128]
        ce = np.cos(w * np.outer(ur, kev)) * kevm[None, :]
        se = np.sin(w * np.outer(ur, kev)) * kevm[None, :]
        co = np.cos(w * np.outer(ur, ko))
        so = np.sin(w * np.outer(ur, ko))
        blocks.append(np.concatenate([ce, se, co, so], axis=1))
    c["tabs"] = np.concatenate(blocks, axis=1).astype(np.float32).astype(BF)

    ce5 = np.cos(w * 512.0 * kev) * kevm
    so5 = np.sin(w * 512.0 * ko)
    c["tab512"] = np.concatenate([ce5, so5]).astype(np.float32).astype(BF)

    # batch indicator for the cnt matmul: ind2[s, i*128+p] = (s//8 == i)
    s_batch = np.arange(SPC) // N
    cols = [np.tile((s_batch == i).astype(np.float32)[:, None], (1, 128))
            for i in range(BPC)]
    c["ind2"] = np.concatenate(cols, axis=1).astype(BF)        # [16, 256]
    return c


# ------------------------------------------------------------------- program
def _build_nc():
    nc = bacc.Bacc("TRN2", target_bir_lowering=False, debug=False,
                   num_devices=NCORES)

    def din(name, shape, dt):
        return nc.dram_tensor(name, shape, dt, kind="ExternalInput").ap()

    MW = 2 * SPC + FEW + 512                      # misc row: xq5 | tab512
    xq = din("xq", [128, 4 * UCH * SPC], BF16)    # butterfly ops a|b|c|d
    misc = din("misc", [1, MW], BF16)
    tabs = din("tabs", [128, UCH * CHW], BF16)    # chunk-major DFT tables
    ltf = din("ltf", [BPC, KTOT, T], BF16)        # onehot28 + 3 conv shifts
    w24 = din("w24", [KCONV, D], BF16)
    r4odd = din("r4odd", [KHOT, 2 * D], F32)
    ind2 = din("ind2", [SPC, BPC * 128], BF16)
    postab = din("postab", [128, NT * D], BF16)
    out = nc.dram_tensor("out", [BPC, 128, NT * D], BF16,
                         kind="ExternalOutput").ap()

    with tile.TileContext(nc) as tc:
        with (
            tc.tile_pool(name="consts", bufs=1) as cpool,
            tc.tile_pool(name="fwork", bufs=1) as fpool,
            tc.tile_pool(name="fpsum", bufs=1, space="PSUM") as fpsum,
            tc.tile_pool(name="mpsum", bufs=3, space="PSUM") as mpsum,
            tc.tile_pool(name="batch", bufs=1) as bpool,
            tc.tile_pool(name="outp", bufs=3) as opool,
            tc.tile_pool(name="cppool", bufs=4) as cppool,
        ):
            UW = UCH * SPC  # 64 butterfly cols

            # -------- SBUF tiles
            xq_sb = fpool.tile([128, 4 * UW], BF16, tag="xq")
            misc_sb = fpool.tile([1, MW], BF16, tag="misc")
            tabs_sb = cpool.tile([128, UCH * CHW], BF16, tag="tabs")
            ind2_sb = cpool.tile([SPC, BPC * 128], BF16, tag="ind2")
            r4odd_sb = cpool.tile([KHOT, 2 * D], F32, tag="r4odd")
            postab_sb = cpool.tile([128, NT * D], BF16, tag="postab")
            lts = [bpool.tile([KTOT, T], BF16, tag=f"lt{i}", name=f"lt{i}")
                   for i in range(BPC)]
            rhss = [bpool.tile([KTOT, D], BF16, tag=f"rhs{i}", name=f"rhs{i}")
                    for i in range(BPC)]
            xq5_sb = misc_sb[:, 0:2 * SPC]
            t512_sb = misc_sb[:, 2 * SPC:]

            # -------- single scalar HWDGE ring in strict priority order
            # (input reads share ~310 GB/s across all queues, so one
            # ordered ring beats parallel rings)
            nc.scalar.dma_start(xq_sb[:], xq)
            nc.scalar.dma_start(misc_sb[:], misc)
            for ch in range(UCH):
                nc.scalar.dma_start(tabs_sb[:, ch * CHW:(ch + 1) * CHW],
                                    tabs[:, ch * CHW:(ch + 1) * CHW])
            nc.scalar.dma_start(lts[0][:], ltf[0])
            nc.scalar.dma_start(lts[1][:], ltf[1])
            nc.scalar.dma_start(ind2_sb[:], ind2)
            nc.scalar.dma_start(r4odd_sb[:], r4odd)
            nc.scalar.dma_start(rhss[0][KHOT:KTOT, :], w24)
            nc.scalar.dma_start(rhss[1][KHOT:KTOT, :], w24)
            for q in range(4):
                nc.scalar.dma_start(
                    postab_sb[:, q * 4 * D:(q + 1) * 4 * D],
                    postab[:, q * 4 * D:(q + 1) * 4 * D])

            # -------- FFT phase: butterflies (vector)
            xa_sb = xq_sb[:, 0 * UW:1 * UW]
            xb_sb = xq_sb[:, 1 * UW:2 * UW]
            xc_sb = xq_sb[:, 2 * UW:3 * UW]
            xd_sb = xq_sb[:, 3 * UW:4 * UW]
            ab = fpool.tile([128, UW], BF16, tag="ab")
            nc.vector.tensor_add(ab[:], xa_sb, xb_sb)
            amb = fpool.tile([128, UW], BF16, tag="amb")
            nc.vector.tensor_sub(amb[:], xa_sb, xb_sb)
            cd = fpool.tile([128, UW], BF16, tag="cd")
            nc.vector.tensor_add(cd[:], xc_sb, xd_sb)
            cmd = fpool.tile([128, UW], BF16, tag="cmd")
            nc.vector.tensor_sub(cmd[:], xc_sb, xd_sb)
            pce = fpool.tile([128, UW], BF16, tag="pce")
            nc.vector.tensor_add(pce[:], ab[:], cd[:])
            pco = fpool.tile([128, UW], BF16, tag="pco")
            nc.vector.tensor_sub(pco[:], ab[:], cd[:])
            pse = fpool.tile([128, UW], BF16, tag="pse")
            nc.vector.tensor_sub(pse[:], amb[:], cmd[:])
            pso = fpool.tile([128, UW], BF16, tag="pso")
            nc.vector.tensor_add(pso[:], amb[:], cmd[:])

            # -------- FFT matmuls: psum bank A rows re_e/re_o/im_e/im_o at
            # bases 0/32/64/96; Nyquist+pad tail in ps_tail
            ps_main = fpsum.tile([112, 512], F32, tag="psmain")
            aux = fpsum.tile([128, 8], F32, tag="aux")
            opnds = {"ce": pce, "co": pco, "se": pse, "so": pso}
            off = {"ce": 0, "se": FEW, "co": 2 * FEW, "so": 2 * FEW + 512}
            for ch in range(UCH):
                st = (ch == 0)
                base = ch * CHW
                for nm, tb in (("ce", 0), ("se", 32)):
                    nc.tensor.matmul(aux[tb:tb + SPC, 0:4],
                                     opnds[nm][:, ch * SPC:(ch + 1) * SPC],
                                     tabs_sb[:, base + off[nm]:base + off[nm] + 4],
                                     start=st, stop=(nm == "se" and ch == UCH - 1),
                                     tile_position=(0, tb),
                                     skip_group_check=True)
                for pb, nm in ((0, "ce"), (32, "co"), (64, "se"), (96, "so")):
                    lhs = opnds[nm][:, ch * SPC:(ch + 1) * SPC]
                    if nm in ("ce", "se"):
                        cols = tabs_sb[:, base + off[nm] + 4:base + off[nm] + FEW]
                    else:
                        cols = tabs_sb[:, base + off[nm]:base + off[nm] + 512]
                    sp = (ch == UCH - 1) and pb in (32, 64)
                    nc.tensor.matmul(ps_main[pb:pb + SPC, :], lhs, cols,
                                     start=st, stop=sp,
                                     tile_position=(0, pb),
                                     skip_group_check=True)
            # u=512 residual: re_e += pce512*cos(pi k/2), im_o += pso512*sin
            nc.tensor.matmul(aux[0:SPC, 0:4], xq5_sb[:, 0:SPC],
                             t512_sb[:, 0:4],
                             start=False, stop=True, tile_position=(0, 0),
                             skip_group_check=True)
            nc.tensor.matmul(ps_main[0:SPC, :], xq5_sb[:, 0:SPC],
                             t512_sb[:, 4:FEW],
                             start=False, stop=True, tile_position=(0, 0),
                             skip_group_check=True)
            nc.tensor.matmul(ps_main[96:96 + SPC, :], xq5_sb[:, SPC:2 * SPC],
                             t512_sb[:, FEW:FEW + 512],
                             start=False, stop=True, tile_position=(0, 96),
                             skip_group_check=True)

            # -------- |X|^2: one big ACT square over all 4 DFT groups,
            # then quarter-aligned DVE adds (re^2 + im^2)
            mag_e = fpool.tile([SPC, 512], F32, tag="mag_e")
            mag_o = fpool.tile([SPC, 512], F32, tag="mag_o")
            mag_t = fpool.tile([SPC, 4], F32, tag="mag_t")
            sqa = fpool.tile([SPC, 512], F32, tag="sqa")
            sqb = fpool.tile([SPC, 512], F32, tag="sqb")
            sqc = fpool.tile([SPC, 4], F32, tag="sqc")
            nc.scalar.square(mag_e[:], ps_main[0:SPC, :])
            nc.scalar.square(sqa[:], ps_main[64:64 + SPC, :])
            nc.vector.tensor_add(mag_e[:], mag_e[:], sqa[:])
            nc.scalar.square(mag_o[:], ps_main[32:32 + SPC, :])
            nc.scalar.square(sqb[:], ps_main[96:96 + SPC, :])
            nc.vector.tensor_add(mag_o[:], mag_o[:], sqb[:])
            nc.scalar.square(mag_t[:], aux[0:SPC, 0:4])
            nc.scalar.square(sqc[:], aux[32:32 + SPC, 0:4])
            nc.vector.tensor_add(mag_t[:], mag_t[:], sqc[:])

            # strict >: Nyquist wins only if greater than every earlier bin
            lm_e = fpool.tile([SPC, 1], F32, tag="lm_e")
            nc.vector.reduce_max(lm_e[:], mag_e[:], axis=mybir.AxisListType.X)
            lm_o = fpool.tile([SPC, 1], F32, tag="lm_o")
            nc.vector.reduce_max(lm_o[:], mag_o[:], axis=mybir.AxisListType.X)
            lm_t = fpool.tile([SPC, 1], F32, tag="lm_t")
            nc.vector.reduce_max(lm_t[:], mag_t[:, 1:4],
                                 axis=mybir.AxisListType.X)
            lm2 = fpool.tile([SPC, 1], F32, tag="lm2")
            nc.vector.tensor_max(lm2[:], lm_e[:], lm_o[:])
            lmax = fpool.tile([SPC, 1], F32, tag="lmax")
            nc.vector.tensor_max(lmax[:], lm2[:], lm_t[:])
            isn = fpool.tile([SPC, 1], BF16, tag="isn")
            nc.vector.tensor_tensor(isn[:], mag_t[:, 0:1], lmax[:],
                                    op=mybir.AluOpType.is_gt)

            a_vecs, bq_vecs = [], []
            for i in range(BPC):
                ps_cnt = aux[:, 4 + i:5 + i]
                nc.tensor.matmul(ps_cnt, ind2_sb[:, i * 128:(i + 1) * 128],
                                 isn[:], start=True, stop=True,
                                 skip_group_check=True)
                a_vec = fpool.tile([128, 1], F32, tag=f"avec{i}")
                nc.vector.tensor_scalar(a_vec[:], ps_cnt, -0.125, 1.0,
                                        op0=mybir.AluOpType.mult,
                                        op1=mybir.AluOpType.add)
                bq_vec = fpool.tile([128, 1], F32, tag=f"bqvec{i}")
                nc.vector.tensor_scalar(bq_vec[:], ps_cnt, 1.0 / 32.0,
                                        None, op0=mybir.AluOpType.mult)
                a_vecs.append(a_vec)
                bq_vecs.append(bq_vec)

            # rhs: hot rows R4 + (cnt/32)*odd; conv rows copied from w24
            for i in range(BPC):
                nc.vector.scalar_tensor_tensor(
                    rhss[i][0:KHOT, :], r4odd_sb[:, D:2 * D],
                    bq_vecs[i][0:KHOT, :], r4odd_sb[:, 0:D],
                    op0=mybir.AluOpType.mult, op1=mybir.AluOpType.add)

            # prescaled a_i*postab for the copy+add drain lanes
            # (only tiles 6..15 of each batch use it)
            aposts = []
            for i in range(BPC):
                ap_t = cpool.tile([128, NT * D], BF16, tag=f"apost{i}",
                                  name=f"apost{i}")
                nc.vector.tensor_scalar(
                    ap_t[:, 0:8 * D], postab_sb[:, 0:8 * D],
                    a_vecs[i][:], None, op0=mybir.AluOpType.mult)
                nc.vector.tensor_scalar(
                    ap_t[:, 8 * D:16 * D], postab_sb[:, 8 * D:16 * D],
                    a_vecs[i][:], None, op0=mybir.AluOpType.mult)
                aposts.append(ap_t)

            # -------- main matmuls (pairs share a 2-bank psum tile) + drain
            # batches interleaved so both drain lanes run concurrently
            for g in range(NT // 4):
                for i in range(BPC):
                    ot4 = opool.tile([128, 4 * D], BF16, tag=f"ot{i}",
                                     name=f"ot{i}")
                    for h in range(2):
                        ps2 = mpsum.tile([128, 2 * D], F32, tag="ps",
                                         name="ps")
                        for s in range(2):
                            ti = g * 4 + h * 2 + s
                            nc.tensor.matmul(
                                ps2[:, s * D:(s + 1) * D],
                                lts[i][:, ti * 128:(ti + 1) * 128],
                                rhss[i][:], start=True, stop=True)
                        pr = g * 2 + h      # pair index 0..7 within batch
                        tlo = (g * 4 + h * 2) * D
                        if pr in (0, 4):
                            # lane A: fused DVE drain straight from PSUM
                            nc.vector.scalar_tensor_tensor(
                                ot4[:, h * 2 * D:(h + 1) * 2 * D],
                                postab_sb[:, tlo:tlo + 2 * D], a_vecs[i][:],
                                ps2[:], op0=mybir.AluOpType.mult,
                                op1=mybir.AluOpType.add)
                        else:
                            # lanes B/C: ACT drains PSUM->bf16, add on
                            # gpsimd (pairs 3-5) or vector (pairs 6-7)
                            cp2 = cppool.tile([128, 2 * D], BF16, tag="cp",
                                              name="cp")
                            nc.scalar.copy(cp2[:], ps2[:])
                            alo = tlo
                            nc.vector.tensor_add(
                                ot4[:, h * 2 * D:(h + 1) * 2 * D],
                                aposts[i][:, alo:alo + 2 * D], cp2[:])
                    nc.sync.dma_start(
                        out[i, :, g * 4 * D:(g + 1) * 4 * D], ot4[:])
    nc.compile()
    return nc


def _get_nc():
    if "nc" not in _cache:
        _cache["nc"] = _build_nc()
    return _cache["nc"]


def _host_inputs(x, x_mark, conv_w):
    # lt rows: 0:28 onehot (hot[b,j,t] = x_mark[b,t,j//7] == j%7),
    #          28:52 three circular shifts of x^T
    xm = np.asarray(x_mark).astype(np.int64)               # [16, 2048, 4]
    j = np.arange(KHOT)
    hot = (xm[:, :, j // 7] == (j % 7)[None, None, :])     # [16, 2048, 28]
    hot = hot.transpose(0, 2, 1).astype(np.float32)        # [16, 28, 2048]
    xt = np.ascontiguousarray(x.transpose(0, 2, 1))        # [16, 8, 2048]
    xtp = np.concatenate([xt[:, :, -1:], xt, xt[:, :, :1]], axis=2)
    ltf = np.concatenate(
        [hot] + [xtp[:, :, k:k + T] for k in range(3)], axis=1)  # [16,52,T]
    ltf = np.ascontiguousarray(ltf).astype(BF)
    # per-core butterfly operands [tt, ch*16 + s], u = ch*128+tt (0..511)
    uu = np.arange(UCH * 128)
    mid = uu >= 1
    ai = uu
    bi = np.where(mid, (T - uu) % T, 0)
    ci = np.where(mid, NYQ - uu, NYQ)
    di = np.where(mid, NYQ + uu, 0)
    dm = mid
    quads = []
    x5 = []
    for core in range(NCORES):
        xs = x[core * BPC:(core + 1) * BPC]                # [2, 2048, 8]
        xflat = xs.transpose(1, 0, 2).reshape(T, SPC)      # [t, s]
        qs = []
        for idx, msk in ((ai, None), (bi, None), (ci, None), (di, dm)):
            arr = xflat[idx]
            if msk is not None:
                arr = arr * msk[:, None]
            qs.append(np.ascontiguousarray(
                arr.reshape(UCH, 128, SPC).transpose(1, 0, 2)
                   .reshape(128, UCH * SPC)).astype(BF))
        quads.append(np.concatenate(qs, axis=1))           # [128, 4*64]
        pce5 = xflat[512] + xflat[1536]
        pso5 = xflat[512] - xflat[1536]
        x5.append(np.concatenate([pce5, pso5]).astype(BF))
    # conv weight rows (k, n): w24[k*8+n, d] = conv_w[d, n, k]
    w24 = np.ascontiguousarray(
        conv_w.transpose(2, 1, 0).reshape(KCONV, D)).astype(BF)
    return ltf, quads, x5, w24


def make_in_maps(x, x_mark, conv_w):
    if "consts" not in _cache:
        _cache["consts"] = _host_constants()
    c = _cache["consts"]
    ltf, quads, x5, w24 = _host_inputs(x, x_mark, conv_w)
    in_maps = []
    for core in range(NCORES):
        b0 = core * BPC
        misc = np.concatenate([x5[core], c["tab512"]])[None, :]
        in_maps.append({
            "xq": quads[core],
            "misc": np.ascontiguousarray(misc),
            "tabs": c["tabs"],
            "ltf": np.ascontiguousarray(ltf[b0:b0 + BPC]),
            "w24": w24,
            "r4odd": c["r4odd"],
            "ind2": c["ind2"],
            "postab": c["postab"],
        })
    return in_maps


# -------------------------------------------------------------------- driver
def kernel(**inputs):
    x = np.asarray(inputs["x"], dtype=np.float32)          # [16, 2048, 8]
    x_mark = np.asarray(inputs["x_mark"])                  # [16, 2048, 4] int
    conv_w = np.asarray(inputs["conv_w"], dtype=np.float32)  # [512, 8, 3]

    in_maps = make_in_maps(x, x_mark, conv_w)
    nc = _get_nc()
    kw = {}
    if TRACE:
        kw = dict(trace=True, tmpdir=TRACE_DIR)
    br = run_bass_kernel_spmd(nc, in_maps, list(range(NCORES)), **kw)
    if TRACE:
        _cache["last_results"] = br

    outp = np.empty((B, T, D), dtype=np.float32)
    for core in range(NCORES):
        o = np.asarray(br.results[core]["out"]).astype(np.float32)
        o = o.reshape(BPC, 128, NT, D).transpose(0, 2, 1, 3).reshape(BPC, T, D)
        outp[core * BPC:(core + 1) * BPC] = o
    return outp
